# revision 27
# baseline (speedup 1.0000x reference)
"""Trainium2 Bass kernel: parallel-beam 3D CT forward projector.

nn_A_55439437856806: x [1,1,256,256,256] f32, angles [128] f32
-> sino [1,1,128,256,256] f32.

Algorithm: out_a[z,u] = sum_t bilinear(V[z], ys(t,u), xs(t,u)) is linear in V,
so per angle it is a sparse banded matrix over the flattened (y,x) plane.
The plane is chunked into 512 fixed 8y x 16x blocks (K=128); per (angle,
chunk) the host prebuilds an fp8 weight matrix [128, W] over the exact
u-window of the band (ragged W per slot/chunk).  On device:
out[z, uoff:uoff+W] += V_chunk[k,z]^T @ M[k, 0:W]  (PE matmuls, PSUM
accumulation; 16 per-angle accumulators fill all 8 PSUM banks).

Sharding: 8 cores x 16 angles (SPMD, one program). Slot a on every core uses
the same compile-time u-window: slot a owns 8 consecutive sorted angles (one
per core) and the window covers the union band over those 8 angles.
"""
import numpy as np
import ml_dtypes

N = 256
HALF = (N - 1) / 2.0
GY, GX = 8, 16
NCY, NCX = N // GY, N // GX
NCHUNK = NCY * NCX          # 512
NCORE = 8
PER = 16                    # angles per core
CG = 8                      # chunks per DMA group
NCG = NCHUNK // CG          # 64
BF16 = ml_dtypes.bfloat16
RDT = ml_dtypes.float8_e4m3   # rhs table dtype
RES_BYTES_PER_PART = 140 * 1024   # SBUF budget for resident R tiles

_RUN_KWARGS = {}            # test harness may set extra run kwargs
_PROG_CACHE = {}


# ---------------------------------------------------------------- host tables

def _angle_samples(theta):
    c, s = np.cos(theta), np.sin(theta)
    t = (np.arange(N, dtype=np.float64) - HALF)
    xs = t[:, None] * c - t[None, :] * s + HALF
    ys = t[:, None] * s + t[None, :] * c + HALF
    x0 = np.floor(xs).astype(np.int64)
    y0 = np.floor(ys).astype(np.int64)
    fx = xs - x0
    fy = ys - y0
    uu = np.broadcast_to(np.arange(N, dtype=np.int64)[None, :], (N, N))
    yis, xis, ws, us = [], [], [], []
    for dy in (0, 1):
        for dx in (0, 1):
            yi = y0 + dy
            xi = x0 + dx
            w = (fx if dx else 1 - fx) * (fy if dy else 1 - fy)
            m = (xi >= 0) & (xi < N) & (yi >= 0) & (yi < N)
            yis.append(yi[m]); xis.append(xi[m]); ws.append(w[m]); us.append(uu[m])
    yi = np.concatenate(yis); xi = np.concatenate(xis)
    w = np.concatenate(ws); u = np.concatenate(us)
    chunk = (yi >> 3) * NCX + (xi >> 4)
    k = (yi & 7) * GX + (xi & 15)
    return chunk, k, u, w


def _build_tables(angles):
    """Returns (order, geom, rhs_cores).

    geom: dict with
      uoff [PER, NCHUNK] window start (int), wwid [PER, NCHUNK] window width
      (0 = slot empty at this chunk), boff [PER, NCHUNK] byte/elem offset of
      the block inside the packed per-chunk-group R row, glen [NCG] packed
      length per chunk group, goff [NCG] start of each group row, rtot.
    rhs_cores: list of NCORE packed R arrays [128, rtot] fp8.
    """
    A = len(angles)
    order = np.argsort(angles, kind="stable")
    samples = [None] * A
    umin = np.full((A, NCHUNK), N * 4, np.int64)
    umax = np.full((A, NCHUNK), -1, np.int64)
    for ai in range(A):
        ch, k, u, w = _angle_samples(float(angles[ai]))
        samples[ai] = (ch, k, u, w)
        np.minimum.at(umin[ai], ch, u)
        np.maximum.at(umax[ai], ch, u)

    uoff = np.zeros((PER, NCHUNK), np.int64)
    wwid = np.zeros((PER, NCHUNK), np.int64)
    for a in range(PER):
        idxs = order[a * NCORE:(a + 1) * NCORE]
        lo = umin[idxs].min(axis=0)
        hi = umax[idxs].max(axis=0)
        ne = hi >= 0
        uoff[a][ne] = lo[ne]
        wwid[a][ne] = hi[ne] - lo[ne] + 1

    aw = wwid
    # pack layout: per chunk group, concat blocks for (i in CG, a in PER)
    boff = np.zeros((PER, NCHUNK), np.int64)
    glen = np.zeros(NCG, np.int64)
    goff = np.zeros(NCG, np.int64)
    pos_all = 0
    for cg in range(NCG):
        goff[cg] = pos_all
        pos = 0
        for i in range(CG):
            c = cg * CG + i
            for a in range(PER):
                if aw[a][c]:
                    boff[a][c] = pos
                    pos += int(aw[a][c])
        glen[cg] = pos
        pos_all += pos
    rtot = int(pos_all)

    rhs_cores = []
    for i in range(NCORE):
        flats, ws = [], []
        for a in range(PER):
            ai = int(order[a * NCORE + i])
            ch, k, u, w = samples[ai]
            base = goff[(ch // CG)] + boff[a][ch]
            j = u - uoff[a][ch]
            ok = (j >= 0) & (j < aw[a][ch])
            assert ok.all()
            flats.append(k * rtot + base + j)
            ws.append(w)
        acc = np.bincount(np.concatenate(flats), weights=np.concatenate(ws),
                          minlength=128 * rtot)
        rhs_cores.append(_quant_dither(acc.reshape(128, rtot).astype(np.float32)))
    geom = dict(uoff=uoff, wwid=aw, boff=boff, glen=glen, goff=goff, rtot=rtot)
    return order, geom, rhs_cores


def _quant_dither(rhs32):
    """fp8 e4m3 with error feedback along k: each row absorbs the previous
    rows' accumulated quantization error, turning bias into noise."""
    if np.dtype(RDT).itemsize != 1:
        return rhs32.astype(RDT)
    q = np.empty_like(rhs32, dtype=RDT)
    carry = np.zeros(rhs32.shape[1], np.float32)
    for k in range(rhs32.shape[0]):
        x = rhs32[k] + carry
        qk = x.astype(RDT)
        q[k] = qk
        carry = x - qk.astype(np.float32)
    return q


def _chunk_volume(vol):
    Z = vol.shape[0]
    v = vol.reshape(Z, NCY, GY, NCX, GX)
    v = v.transpose(1, 3, 2, 4, 0)
    return np.ascontiguousarray(v.reshape(NCHUNK, 128, Z))


# ---------------------------------------------------------------- bass kernel

def _geom_key(geom):
    return (geom["uoff"].tobytes(), geom["wwid"].tobytes())


def _build_nc(geom, n_cg=NCG, halves=(0, 1)):
    import dataclasses
    import concourse.bacc as bacc
    import concourse.mybir as mybir
    import concourse.tile as tile

    f32 = mybir.dt.float32
    bf16 = mybir.dt.bfloat16
    rdt = mybir.dt.from_np(np.dtype(RDT))
    uoff, wwid, boff = geom["uoff"], geom["wwid"], geom["boff"]
    glen, goff, rtot = geom["glen"], geom["goff"], geom["rtot"]
    glen_max = int(glen.max())

    # how many chunk-group R tiles stay resident across the two z-halves;
    # budget = SBUF per partition minus the working pools (vp/rp/ep)
    rsize = int(np.dtype(RDT).itemsize)
    vp_bufs, rp_bufs = 6, 4
    working = vp_bufs * CG * 128 * 2 + rp_bufs * glen_max * rsize + 2 * N * 4
    res_bytes = min(RES_BYTES_PER_PART, 186 * 1024 - working)
    budget = max(res_bytes, 0) // rsize
    n_res = 0
    acc = 0
    for cg in range(n_cg):
        if acc + int(glen[cg]) > budget:
            break
        acc += int(glen[cg])
        n_res = cg + 1

    nc = bacc.Bacc("TRN2", target_bir_lowering=False, debug=False)
    vd = nc.dram_tensor("v", [2, NCG, 128, CG * 128], bf16, kind="ExternalInput")
    rd = nc.dram_tensor("r", [128, rtot], rdt, kind="ExternalInput")
    od = nc.dram_tensor("o", [PER, 2, 128, N], f32, kind="ExternalOutput")
    vap, rap, oap = vd.ap(), rd.ap(), od.ap()

    with tile.TileContext(nc) as tc:
        with (
            tc.tile_pool(name="vp", bufs=vp_bufs) as vp,
            tc.tile_pool(name="rp", bufs=rp_bufs) as rp,
            tc.tile_pool(name="ep", bufs=2) as ep,
            tc.tile_pool(name="rres", bufs=1) as rres,
            tc.tile_pool(name="pp", bufs=1, space="PSUM") as pp,
        ):
            psum = [pp.tile([128, 512], f32, tag=f"ps{b}", name=f"ps{b}")
                    for b in range(8)]
            res_tiles = {}
            for h in halves:
                for b in range(8):
                    nc.vector.memset(psum[b][:], 0.0)
                for cg in range(n_cg):
                    L = int(glen[cg])
                    if L == 0:
                        continue
                    vt = vp.tile([128, CG * 128], bf16, tag="v", name="vt")
                    nc.sync.dma_start(vt[:], vap[h, cg])
                    if cg < n_res:
                        if cg in res_tiles:
                            rt = res_tiles[cg]          # reuse, no DMA
                        else:
                            rt = rres.tile([128, L], rdt, tag=f"rr{cg}",
                                           name=f"rr{cg}")
                            res_tiles[cg] = rt
                            nc.scalar.dma_start(
                                rt[:], rap[:, int(goff[cg]):int(goff[cg]) + L])
                    else:
                        rt = rp.tile([128, glen_max], rdt, tag="r", name="rt")
                        nc.scalar.dma_start(
                            rt[:, 0:L], rap[:, int(goff[cg]):int(goff[cg]) + L])
                    for i in range(CG):
                        c = cg * CG + i
                        for a in range(PER):
                            Wc = int(wwid[a][c])
                            if Wc == 0:
                                continue
                            off = (a % 2) * 256 + int(uoff[a][c])
                            b0 = int(boff[a][c])
                            nc.tensor.matmul(
                                psum[a // 2][:, off:off + Wc],
                                vt[:, i * 128:(i + 1) * 128],
                                rt[:, b0:b0 + Wc],
                                start=False, stop=False,
                                skip_group_check=True,
                            )
                for a in range(PER):
                    et = ep.tile([128, N], f32, tag="e", name="et")
                    nc.vector.tensor_copy(
                        et[:], psum[a // 2][:, (a % 2) * 256:(a % 2) * 256 + N])
                    nc.sync.dma_start(oap[a, h], et[:])
    nc.compile()
    return nc


# ---------------------------------------------------------------- entrypoint

def kernel(x, angles):
    from concourse import bass_utils

    x = np.asarray(x)
    angles = np.asarray(angles)
    order, geom, rhs_cores = _build_tables(angles)

    vol = np.ascontiguousarray(x[0, 0]).astype(np.float32)
    Vc = _chunk_volume(vol).astype(BF16)                   # [512, 128, 256]
    vdn = Vc.reshape(NCG, CG, 128, 2, 128).transpose(3, 0, 2, 1, 4)
    vdn = np.ascontiguousarray(vdn).reshape(2, NCG, 128, CG * 128)

    key = _geom_key(geom)
    if key not in _PROG_CACHE:
        _PROG_CACHE[key] = _build_nc(geom)
    nc = _PROG_CACHE[key]

    in_maps = [{"v": vdn, "r": rhs_cores[i]} for i in range(NCORE)]
    res = bass_utils.run_bass_kernel_spmd(
        nc, in_maps, core_ids=list(range(NCORE)), **_RUN_KWARGS
    )

    out = np.zeros((len(angles), 256, N), np.float32)
    for i in range(NCORE):
        o = res.results[i]["o"]                            # [PER, 2, 128, N]
        for a in range(PER):
            ai = int(order[a * NCORE + i])
            out[ai, 0:128] = o[a, 0]
            out[ai, 128:256] = o[a, 1]
    kernel.last_results = res
    return out.reshape(1, 1, len(angles), 256, N)


# revision 33
# speedup vs baseline: 1.1261x; 1.1261x over previous
"""Trainium2 Bass kernel: parallel-beam 3D CT forward projector.

nn_A_55439437856806: x [1,1,256,256,256] f32, angles [128] f32
-> sino [1,1,128,256,256] f32.

Algorithm: out_a[z,u] = sum_t bilinear(V[z], ys(t,u), xs(t,u)) is linear in V,
so per angle it is a sparse banded matrix over the flattened (y,x) plane.
The plane is chunked into 512 fixed 8y x 16x blocks (K=128); per (angle,
chunk) the host prebuilds an fp8 weight matrix [128, W] over the exact
u-window of the band (ragged W per slot/chunk).  On device:
out[z, uoff:uoff+W] += V_chunk[k,z]^T @ M[k, 0:W]  (PE matmuls, PSUM
accumulation; 16 per-angle accumulators fill all 8 PSUM banks).

Sharding: 8 cores x 16 angles (SPMD, one program). Slot a on every core uses
the same compile-time u-window: slot a owns 8 consecutive sorted angles (one
per core) and the window covers the union band over those 8 angles.
"""
import numpy as np
import ml_dtypes

N = 256
HALF = (N - 1) / 2.0
GY, GX = 8, 16
NCY, NCX = N // GY, N // GX
NCHUNK = NCY * NCX          # 512
NCORE = 8
PER = 16                    # angles per core
CG = 8                      # chunks per DMA group
NCG = NCHUNK // CG          # 64
BF16 = ml_dtypes.bfloat16
RDT = ml_dtypes.float8_e4m3   # rhs table dtype
VDT = ml_dtypes.float8_e4m3   # volume dtype
RES_BYTES_PER_PART = 140 * 1024   # SBUF budget for resident R tiles

_RUN_KWARGS = {}            # test harness may set extra run kwargs
_PROG_CACHE = {}


# ---------------------------------------------------------------- host tables

def _angle_samples(theta):
    c, s = np.cos(theta), np.sin(theta)
    t = (np.arange(N, dtype=np.float64) - HALF)
    xs = t[:, None] * c - t[None, :] * s + HALF
    ys = t[:, None] * s + t[None, :] * c + HALF
    x0 = np.floor(xs).astype(np.int64)
    y0 = np.floor(ys).astype(np.int64)
    fx = xs - x0
    fy = ys - y0
    uu = np.broadcast_to(np.arange(N, dtype=np.int64)[None, :], (N, N))
    yis, xis, ws, us = [], [], [], []
    for dy in (0, 1):
        for dx in (0, 1):
            yi = y0 + dy
            xi = x0 + dx
            w = (fx if dx else 1 - fx) * (fy if dy else 1 - fy)
            m = (xi >= 0) & (xi < N) & (yi >= 0) & (yi < N)
            yis.append(yi[m]); xis.append(xi[m]); ws.append(w[m]); us.append(uu[m])
    yi = np.concatenate(yis); xi = np.concatenate(xis)
    w = np.concatenate(ws); u = np.concatenate(us)
    chunk = (yi >> 3) * NCX + (xi >> 4)
    k = (yi & 7) * GX + (xi & 15)
    return chunk, k, u, w


def _build_tables(angles):
    """Returns (order, geom, rhs_cores).

    geom: dict with
      uoff [PER, NCHUNK] window start (int), wwid [PER, NCHUNK] window width
      (0 = slot empty at this chunk), boff [PER, NCHUNK] byte/elem offset of
      the block inside the packed per-chunk-group R row, glen [NCG] packed
      length per chunk group, goff [NCG] start of each group row, rtot.
    rhs_cores: list of NCORE packed R arrays [128, rtot] fp8.
    """
    A = len(angles)
    order = np.argsort(angles, kind="stable")
    samples = [None] * A
    umin = np.full((A, NCHUNK), N * 4, np.int64)
    umax = np.full((A, NCHUNK), -1, np.int64)
    for ai in range(A):
        ch, k, u, w = _angle_samples(float(angles[ai]))
        samples[ai] = (ch, k, u, w)
        np.minimum.at(umin[ai], ch, u)
        np.maximum.at(umax[ai], ch, u)

    uoff = np.zeros((PER, NCHUNK), np.int64)
    wwid = np.zeros((PER, NCHUNK), np.int64)
    for a in range(PER):
        idxs = order[a * NCORE:(a + 1) * NCORE]
        lo = umin[idxs].min(axis=0)
        hi = umax[idxs].max(axis=0)
        ne = hi >= 0
        uoff[a][ne] = lo[ne]
        wwid[a][ne] = hi[ne] - lo[ne] + 1

    aw = wwid
    # pack layout: per chunk group, concat blocks for (i in CG, a in PER)
    boff = np.zeros((PER, NCHUNK), np.int64)
    glen = np.zeros(NCG, np.int64)
    goff = np.zeros(NCG, np.int64)
    pos_all = 0
    for cg in range(NCG):
        goff[cg] = pos_all
        pos = 0
        for i in range(CG):
            c = cg * CG + i
            for a in range(PER):
                if aw[a][c]:
                    boff[a][c] = pos
                    pos += int(aw[a][c])
        glen[cg] = pos
        pos_all += pos
    rtot = int(pos_all)

    rhs_cores = []
    for i in range(NCORE):
        flats, ws = [], []
        for a in range(PER):
            ai = int(order[a * NCORE + i])
            ch, k, u, w = samples[ai]
            base = goff[(ch // CG)] + boff[a][ch]
            j = u - uoff[a][ch]
            ok = (j >= 0) & (j < aw[a][ch])
            assert ok.all()
            flats.append(k * rtot + base + j)
            ws.append(w)
        acc = np.bincount(np.concatenate(flats), weights=np.concatenate(ws),
                          minlength=128 * rtot)
        rhs_cores.append(_quant_dither(acc.reshape(128, rtot).astype(np.float32)))
    geom = dict(uoff=uoff, wwid=aw, boff=boff, glen=glen, goff=goff, rtot=rtot)
    return order, geom, rhs_cores


def _quant_dither(rhs32):
    """fp8 e4m3 with error feedback along k: each row absorbs the previous
    rows' accumulated quantization error, turning bias into noise."""
    if np.dtype(RDT).itemsize != 1:
        return rhs32.astype(RDT)
    q = np.empty_like(rhs32, dtype=RDT)
    carry = np.zeros(rhs32.shape[1], np.float32)
    for k in range(rhs32.shape[0]):
        x = rhs32[k] + carry
        qk = x.astype(RDT)
        q[k] = qk
        carry = x - qk.astype(np.float32)
    return q


def _chunk_volume(vol):
    Z = vol.shape[0]
    v = vol.reshape(Z, NCY, GY, NCX, GX)
    v = v.transpose(1, 3, 2, 4, 0)
    return np.ascontiguousarray(v.reshape(NCHUNK, 128, Z))


# ---------------------------------------------------------------- bass kernel

def _geom_key(geom):
    return (geom["uoff"].tobytes(), geom["wwid"].tobytes())


def _build_nc(geom, n_cg=NCG, halves=(0, 1)):
    import dataclasses
    import concourse.bacc as bacc
    import concourse.mybir as mybir
    import concourse.tile as tile

    f32 = mybir.dt.float32
    vdt = mybir.dt.from_np(np.dtype(VDT))
    rdt = mybir.dt.from_np(np.dtype(RDT))
    uoff, wwid, boff = geom["uoff"], geom["wwid"], geom["boff"]
    glen, goff, rtot = geom["glen"], geom["goff"], geom["rtot"]
    glen_max = int(glen.max())

    # how many chunk-group R tiles stay resident across the two z-halves;
    # budget = SBUF per partition minus the working pools (vp/rp/ep)
    rsize = int(np.dtype(RDT).itemsize)
    vsize = int(np.dtype(VDT).itemsize)
    vp_bufs, rp_bufs = 6, 4
    working = vp_bufs * CG * 128 * vsize + rp_bufs * glen_max * rsize + 2 * N * 4
    res_bytes = min(RES_BYTES_PER_PART, 186 * 1024 - working)
    budget = max(res_bytes, 0) // rsize
    n_res = 0
    acc = 0
    for cg in range(n_cg):
        if acc + int(glen[cg]) > budget:
            break
        acc += int(glen[cg])
        n_res = cg + 1

    nc = bacc.Bacc("TRN2", target_bir_lowering=False, debug=False)
    vd = nc.dram_tensor("v", [2, NCG, 128, CG * 128], vdt, kind="ExternalInput")
    rd = nc.dram_tensor("r", [128, rtot], rdt, kind="ExternalInput")
    od = nc.dram_tensor("o", [PER, 2, 128, N], f32, kind="ExternalOutput")
    vap, rap, oap = vd.ap(), rd.ap(), od.ap()

    with tile.TileContext(nc) as tc:
        with (
            tc.tile_pool(name="vp", bufs=vp_bufs) as vp,
            tc.tile_pool(name="rp", bufs=rp_bufs) as rp,
            tc.tile_pool(name="ep", bufs=2) as ep,
            tc.tile_pool(name="rres", bufs=1) as rres,
            tc.tile_pool(name="pp", bufs=1, space="PSUM") as pp,
        ):
            psum = [pp.tile([128, 512], f32, tag=f"ps{b}", name=f"ps{b}")
                    for b in range(8)]
            res_tiles = {}
            for h in halves:
                for b in range(8):
                    nc.vector.memset(psum[b][:], 0.0)
                for cg in range(n_cg):
                    L = int(glen[cg])
                    if L == 0:
                        continue
                    vt = vp.tile([128, CG * 128], vdt, tag="v", name="vt")
                    nc.sync.dma_start(vt[:], vap[h, cg])
                    if cg < n_res:
                        if cg in res_tiles:
                            rt = res_tiles[cg]          # reuse, no DMA
                        else:
                            rt = rres.tile([128, L], rdt, tag=f"rr{cg}",
                                           name=f"rr{cg}")
                            res_tiles[cg] = rt
                            nc.scalar.dma_start(
                                rt[:], rap[:, int(goff[cg]):int(goff[cg]) + L])
                    else:
                        rt = rp.tile([128, glen_max], rdt, tag="r", name="rt")
                        nc.scalar.dma_start(
                            rt[:, 0:L], rap[:, int(goff[cg]):int(goff[cg]) + L])
                    for i in range(CG):
                        c = cg * CG + i
                        for a in range(PER):
                            Wc = int(wwid[a][c])
                            if Wc == 0:
                                continue
                            off = (a % 2) * 256 + int(uoff[a][c])
                            b0 = int(boff[a][c])
                            nc.tensor.matmul(
                                psum[a // 2][:, off:off + Wc],
                                vt[:, i * 128:(i + 1) * 128],
                                rt[:, b0:b0 + Wc],
                                start=False, stop=False,
                                skip_group_check=True,
                            )
                for a in range(PER):
                    et = ep.tile([128, N], f32, tag="e", name="et")
                    nc.vector.tensor_copy(
                        et[:], psum[a // 2][:, (a % 2) * 256:(a % 2) * 256 + N])
                    nc.sync.dma_start(oap[a, h], et[:])
    nc.compile()
    return nc


# ---------------------------------------------------------------- entrypoint

def kernel(x, angles):
    from concourse import bass_utils

    x = np.asarray(x)
    angles = np.asarray(angles)
    order, geom, rhs_cores = _build_tables(angles)

    vol = np.ascontiguousarray(x[0, 0]).astype(np.float32)
    Vc = _chunk_volume(vol).astype(VDT)                    # [512, 128, 256]
    vdn = Vc.reshape(NCG, CG, 128, 2, 128).transpose(3, 0, 2, 1, 4)
    vdn = np.ascontiguousarray(vdn).reshape(2, NCG, 128, CG * 128)

    key = _geom_key(geom)
    if key not in _PROG_CACHE:
        _PROG_CACHE[key] = _build_nc(geom)
    nc = _PROG_CACHE[key]

    in_maps = [{"v": vdn, "r": rhs_cores[i]} for i in range(NCORE)]
    res = bass_utils.run_bass_kernel_spmd(
        nc, in_maps, core_ids=list(range(NCORE)), **_RUN_KWARGS
    )

    out = np.zeros((len(angles), 256, N), np.float32)
    for i in range(NCORE):
        o = res.results[i]["o"]                            # [PER, 2, 128, N]
        for a in range(PER):
            ai = int(order[a * NCORE + i])
            out[ai, 0:128] = o[a, 0]
            out[ai, 128:256] = o[a, 1]
    kernel.last_results = res
    return out.reshape(1, 1, len(angles), 256, N)


# revision 38
# speedup vs baseline: 1.1404x; 1.0127x over previous
"""Trainium2 Bass kernel: parallel-beam 3D CT forward projector.

nn_A_55439437856806: x [1,1,256,256,256] f32, angles [128] f32
-> sino [1,1,128,256,256] f32.

Algorithm: out_a[z,u] = sum_t bilinear(V[z], ys(t,u), xs(t,u)) is linear in V,
so per angle it is a sparse banded matrix over the flattened (y,x) plane.
The plane is chunked into 512 fixed 8y x 16x blocks (K=128); per (angle,
chunk) the host prebuilds an fp8 weight matrix [128, W] over the exact
u-window of the band (ragged W per slot/chunk).  On device:
out[z, uoff:uoff+W] += V_chunk[k,z]^T @ M[k, 0:W]  (PE matmuls, PSUM
accumulation; 16 per-angle accumulators fill all 8 PSUM banks).

Sharding: 8 cores x 16 angles (SPMD, one program). Slot a on every core uses
the same compile-time u-window: slot a owns 8 consecutive sorted angles (one
per core) and the window covers the union band over those 8 angles.
"""
import numpy as np
import ml_dtypes

N = 256
HALF = (N - 1) / 2.0
GY, GX = 8, 16
NCY, NCX = N // GY, N // GX
NCHUNK = NCY * NCX          # 512
NCORE = 8
PER = 16                    # angles per core
CG = 8                      # chunks per DMA group
NCG = NCHUNK // CG          # 64
BF16 = ml_dtypes.bfloat16
RDT = ml_dtypes.float8_e4m3   # rhs table dtype
VDT = ml_dtypes.float8_e4m3   # volume dtype
RES_BYTES_PER_PART = 158 * 1024   # SBUF budget for resident R tiles

_RUN_KWARGS = {}            # test harness may set extra run kwargs
_PROG_CACHE = {}


# ---------------------------------------------------------------- host tables

def _angle_samples(theta):
    c, s = np.cos(theta), np.sin(theta)
    t = (np.arange(N, dtype=np.float64) - HALF)
    xs = t[:, None] * c - t[None, :] * s + HALF
    ys = t[:, None] * s + t[None, :] * c + HALF
    x0 = np.floor(xs).astype(np.int64)
    y0 = np.floor(ys).astype(np.int64)
    fx = xs - x0
    fy = ys - y0
    uu = np.broadcast_to(np.arange(N, dtype=np.int64)[None, :], (N, N))
    yis, xis, ws, us = [], [], [], []
    for dy in (0, 1):
        for dx in (0, 1):
            yi = y0 + dy
            xi = x0 + dx
            w = (fx if dx else 1 - fx) * (fy if dy else 1 - fy)
            m = (xi >= 0) & (xi < N) & (yi >= 0) & (yi < N) & (w != 0)
            yis.append(yi[m]); xis.append(xi[m]); ws.append(w[m]); us.append(uu[m])
    yi = np.concatenate(yis); xi = np.concatenate(xis)
    w = np.concatenate(ws); u = np.concatenate(us)
    chunk = (yi >> 3) * NCX + (xi >> 4)
    k = (yi & 7) * GX + (xi & 15)
    return chunk, k, u, w


def _build_tables(angles):
    """Returns (order, geom, rhs_cores).

    geom: dict with
      uoff [PER, NCHUNK] window start (int), wwid [PER, NCHUNK] window width
      (0 = slot empty at this chunk), boff [PER, NCHUNK] byte/elem offset of
      the block inside the packed per-chunk-group R row, glen [NCG] packed
      length per chunk group, goff [NCG] start of each group row, rtot.
    rhs_cores: list of NCORE packed R arrays [128, rtot] fp8.
    """
    A = len(angles)
    order = np.argsort(angles, kind="stable")
    samples = [None] * A
    umin = np.full((A, NCHUNK), N * 4, np.int64)
    umax = np.full((A, NCHUNK), -1, np.int64)
    for ai in range(A):
        ch, k, u, w = _angle_samples(float(angles[ai]))
        samples[ai] = (ch, k, u, w)
        np.minimum.at(umin[ai], ch, u)
        np.maximum.at(umax[ai], ch, u)

    uoff = np.zeros((PER, NCHUNK), np.int64)
    wwid = np.zeros((PER, NCHUNK), np.int64)
    for a in range(PER):
        idxs = order[a * NCORE:(a + 1) * NCORE]
        lo = umin[idxs].min(axis=0)
        hi = umax[idxs].max(axis=0)
        ne = hi >= 0
        uoff[a][ne] = lo[ne]
        wwid[a][ne] = hi[ne] - lo[ne] + 1

    aw = wwid
    # pack layout: per chunk group, concat blocks for (i in CG, a in PER)
    boff = np.zeros((PER, NCHUNK), np.int64)
    glen = np.zeros(NCG, np.int64)
    goff = np.zeros(NCG, np.int64)
    pos_all = 0
    for cg in range(NCG):
        goff[cg] = pos_all
        pos = 0
        for i in range(CG):
            c = cg * CG + i
            for a in range(PER):
                if aw[a][c]:
                    boff[a][c] = pos
                    pos += int(aw[a][c])
        glen[cg] = pos
        pos_all += pos
    rtot = int(pos_all)

    rhs_cores = []
    for i in range(NCORE):
        flats, ws = [], []
        for a in range(PER):
            ai = int(order[a * NCORE + i])
            ch, k, u, w = samples[ai]
            base = goff[(ch // CG)] + boff[a][ch]
            j = u - uoff[a][ch]
            ok = (j >= 0) & (j < aw[a][ch])
            assert ok.all()
            flats.append(k * rtot + base + j)
            ws.append(w)
        acc = np.bincount(np.concatenate(flats), weights=np.concatenate(ws),
                          minlength=128 * rtot)
        rhs_cores.append(_quant_dither(acc.reshape(128, rtot).astype(np.float32)))
    geom = dict(uoff=uoff, wwid=aw, boff=boff, glen=glen, goff=goff, rtot=rtot)
    return order, geom, rhs_cores


def _quant_dither(rhs32):
    """fp8 e4m3 with error feedback along k: each row absorbs the previous
    rows' accumulated quantization error, turning bias into noise."""
    if np.dtype(RDT).itemsize != 1:
        return rhs32.astype(RDT)
    q = np.empty_like(rhs32, dtype=RDT)
    carry = np.zeros(rhs32.shape[1], np.float32)
    for k in range(rhs32.shape[0]):
        x = rhs32[k] + carry
        qk = x.astype(RDT)
        q[k] = qk
        carry = x - qk.astype(np.float32)
    return q


def _chunk_volume(vol):
    Z = vol.shape[0]
    v = vol.reshape(Z, NCY, GY, NCX, GX)
    v = v.transpose(1, 3, 2, 4, 0)
    return np.ascontiguousarray(v.reshape(NCHUNK, 128, Z))


# ---------------------------------------------------------------- bass kernel

def _geom_key(geom):
    return (geom["uoff"].tobytes(), geom["wwid"].tobytes())


def _build_nc(geom, n_cg=NCG, halves=(0, 1)):
    import dataclasses
    import concourse.bacc as bacc
    import concourse.mybir as mybir
    import concourse.tile as tile

    f32 = mybir.dt.float32
    vdt = mybir.dt.from_np(np.dtype(VDT))
    rdt = mybir.dt.from_np(np.dtype(RDT))
    uoff, wwid, boff = geom["uoff"], geom["wwid"], geom["boff"]
    glen, goff, rtot = geom["glen"], geom["goff"], geom["rtot"]
    glen_max = int(glen.max())

    # how many chunk-group R tiles stay resident across the two z-halves;
    # budget = SBUF per partition minus the working pools (vp/rp/ep)
    rsize = int(np.dtype(RDT).itemsize)
    vsize = int(np.dtype(VDT).itemsize)
    vp_bufs, rp_bufs = 8, 5
    working = vp_bufs * CG * 128 * vsize + rp_bufs * glen_max * rsize + 2 * N * 4
    res_bytes = min(RES_BYTES_PER_PART, 186 * 1024 - working)
    budget = max(res_bytes, 0) // rsize
    n_res = 0
    acc = 0
    for cg in range(n_cg):
        if acc + int(glen[cg]) > budget:
            break
        acc += int(glen[cg])
        n_res = cg + 1

    nc = bacc.Bacc("TRN2", target_bir_lowering=False, debug=False)
    vd = nc.dram_tensor("v", [2, NCG, 128, CG * 128], vdt, kind="ExternalInput")
    rd = nc.dram_tensor("r", [128, rtot], rdt, kind="ExternalInput")
    od = nc.dram_tensor("o", [PER, 2, 128, N], f32, kind="ExternalOutput")
    vap, rap, oap = vd.ap(), rd.ap(), od.ap()

    with tile.TileContext(nc) as tc:
        with (
            tc.tile_pool(name="vp", bufs=vp_bufs) as vp,
            tc.tile_pool(name="rp", bufs=rp_bufs) as rp,
            tc.tile_pool(name="ep", bufs=2) as ep,
            tc.tile_pool(name="rres", bufs=1) as rres,
            tc.tile_pool(name="pp", bufs=1, space="PSUM") as pp,
        ):
            psum = [pp.tile([128, 512], f32, tag=f"ps{b}", name=f"ps{b}")
                    for b in range(8)]
            res_tiles = {}
            for h in halves:
                for b in range(8):
                    nc.vector.memset(psum[b][:], 0.0)
                for cg in range(n_cg):
                    L = int(glen[cg])
                    if L == 0:
                        continue
                    vt = vp.tile([128, CG * 128], vdt, tag="v", name="vt")
                    nc.sync.dma_start(vt[:], vap[h, cg])
                    if cg < n_res:
                        if cg in res_tiles:
                            rt = res_tiles[cg]          # reuse, no DMA
                        else:
                            rt = rres.tile([128, L], rdt, tag=f"rr{cg}",
                                           name=f"rr{cg}")
                            res_tiles[cg] = rt
                            nc.scalar.dma_start(
                                rt[:], rap[:, int(goff[cg]):int(goff[cg]) + L])
                    else:
                        rt = rp.tile([128, glen_max], rdt, tag="r", name="rt")
                        nc.scalar.dma_start(
                            rt[:, 0:L], rap[:, int(goff[cg]):int(goff[cg]) + L])
                    for i in range(CG):
                        c = cg * CG + i
                        for a in range(PER):
                            Wc = int(wwid[a][c])
                            if Wc == 0:
                                continue
                            off = (a % 2) * 256 + int(uoff[a][c])
                            b0 = int(boff[a][c])
                            nc.tensor.matmul(
                                psum[a // 2][:, off:off + Wc],
                                vt[:, i * 128:(i + 1) * 128],
                                rt[:, b0:b0 + Wc],
                                start=False, stop=False,
                                skip_group_check=True,
                            )
                for a in range(PER):
                    et = ep.tile([128, N], f32, tag="e", name="et")
                    nc.vector.tensor_copy(
                        et[:], psum[a // 2][:, (a % 2) * 256:(a % 2) * 256 + N])
                    nc.sync.dma_start(oap[a, h], et[:])
    nc.compile()
    return nc


# ---------------------------------------------------------------- entrypoint

def kernel(x, angles):
    from concourse import bass_utils

    x = np.asarray(x)
    angles = np.asarray(angles)
    order, geom, rhs_cores = _build_tables(angles)

    vol = np.ascontiguousarray(x[0, 0]).astype(np.float32)
    Vc = _chunk_volume(vol).astype(VDT)                    # [512, 128, 256]
    vdn = Vc.reshape(NCG, CG, 128, 2, 128).transpose(3, 0, 2, 1, 4)
    vdn = np.ascontiguousarray(vdn).reshape(2, NCG, 128, CG * 128)

    key = _geom_key(geom)
    if key not in _PROG_CACHE:
        _PROG_CACHE[key] = _build_nc(geom)
    nc = _PROG_CACHE[key]

    in_maps = [{"v": vdn, "r": rhs_cores[i]} for i in range(NCORE)]
    res = bass_utils.run_bass_kernel_spmd(
        nc, in_maps, core_ids=list(range(NCORE)), **_RUN_KWARGS
    )

    out = np.zeros((len(angles), 256, N), np.float32)
    for i in range(NCORE):
        o = res.results[i]["o"]                            # [PER, 2, 128, N]
        for a in range(PER):
            ai = int(order[a * NCORE + i])
            out[ai, 0:128] = o[a, 0]
            out[ai, 128:256] = o[a, 1]
    kernel.last_results = res
    return out.reshape(1, 1, len(angles), 256, N)
